# revision 82
# baseline (speedup 1.0000x reference)
"""Trainium2 Bass kernel for nn_MultiHeadAttn (conv-QKV multi-head attention).

Sharding: pure data parallelism over batch B=8 -> one batch item per NeuronCore.

Per-core pipeline (matmuls bf16; cost model charges N_out_cols x 1 cycle):
  - 3x3 SAME convs via Winograd F(2x2,3x3): 16 transformed planes, each a
    [1024ic x 1024oc] GEMM over 256 tiles -> 16*8*8 matmuls of N=256 per conv
    (2.25x fewer PE columns than direct conv).  Weights host-transformed
    (G W G^T) to bf16.
  - Precision: conv error is amplified ~4x through exp(logits/8) in the
    attention, so the transform chain keeps fp32 intermediates everywhere;
    the only roundings are x->bf16, T->bf16, U->bf16, y->bf16.  Input
    transform: stage B (columns, on host-deinterleaved even/odd planes,
    all reads stride-1) bf16->fp32, stage C (rows) fp32->bf16 on DVE.
    Output transform stage D runs on DVE straight out of PSUM with fp32
    temps into bf16 P; stage E (fp32 temps -> bf16 y) runs on Pool.
  - Conv bias folded in as a K=1 matmul into the (i=1,j=1) plane (its output
    transform coefficient is +1 for every output pixel).
  - Feature pixels are stored in a head-preserving permutation
    f' = ty*64 + a*32 + b*16 + tx (head == tile row ty), applied
    consistently to Q/K/V and Wo's input dim, so stage E writes stay packed.
  - Attention per (head, k-chunk): S^T = K Q^T (bf16), exp on ACT with
    scale=1/8 straight from PSUM to bf16, mask applied POST-exp as a bf16
    multiply on DVE (mask in {0,1}), PV matmul bf16 with a ones column
    appended to V so softmax denominators fall out of the PV matmul.
  - Normalization: fp32 reciprocal of the denominator row, DRAM-bounce
    partition-broadcast, multiply into bf16 O^T staging.
  - Output linear computed TRANSPOSED (out^T[j,t] = Wo O^T) so the bias is
    per-partition; host transposes the result back (layout only).
Host-side work is layout/cast plus weight transform (weight preprocessing).
"""

import sys

if "/opt/trn_rl_repo" not in sys.path:
    sys.path.insert(0, "/opt/trn_rl_repo")

import numpy as np

_CACHE = {}

B = 8
C = 1024          # tokens (= conv channels)
NH = 16           # heads
HD = 64           # head dim
XL = 1156         # deinterleaved padded plane: [2 eo][34 y][17 x]

# f' permutation: perm[f'] = original f, f' = ty*64 + a*32 + b*16 + tx
_PERM = np.zeros(1024, np.int64)
for _ty in range(16):
    for _a in range(2):
        for _b in range(2):
            for _tx in range(16):
                _PERM[_ty * 64 + _a * 32 + _b * 16 + _tx] = \
                    (2 * _ty + _a) * 32 + 2 * _tx + _b


def _build_program(reps=1):
    from contextlib import ExitStack

    import concourse.bass as bass
    import concourse.mybir as mybir
    import concourse.tile as tile
    from concourse import bacc

    FP = mybir.dt.float32
    BF = mybir.dt.bfloat16

    nc = bacc.Bacc(None, target_bir_lowering=False)

    xq_d = nc.dram_tensor("xq", [C, XL], BF, kind="ExternalInput")
    xk_d = nc.dram_tensor("xk", [C, XL], BF, kind="ExternalInput")
    xv_d = nc.dram_tensor("xv", [C, XL], BF, kind="ExternalInput")
    wq_d = nc.dram_tensor("wq", [16, 2, 128, 8, 4, 128], BF, kind="ExternalInput")
    wk_d = nc.dram_tensor("wk", [16, 2, 128, 8, 4, 128], BF, kind="ExternalInput")
    wv_d = nc.dram_tensor("wv", [16, 2, 128, 8, 4, 128], BF, kind="ExternalInput")
    bq_d = nc.dram_tensor("bq", [1, C], BF, kind="ExternalInput")
    bk_d = nc.dram_tensor("bk", [1, C], BF, kind="ExternalInput")
    bv_d = nc.dram_tensor("bv", [1, C], BF, kind="ExternalInput")
    wo_d = nc.dram_tensor("wo", [C, C], BF, kind="ExternalInput")   # Wo^T[f', j]
    bo_d = nc.dram_tensor("bo", [C], FP, kind="ExternalInput")
    mt_d = nc.dram_tensor("mt", [C, C], BF, kind="ExternalInput")   # mask^T [s, t]
    out_d = nc.dram_tensor("out", [C, C], FP, kind="ExternalOutput")  # out^T [j, t]

    with ExitStack() as ctx:
        tc = ctx.enter_context(tile.TileContext(nc))
        for _rep in range(reps):
            _build_body(nc, tc, bass, mybir, tile,
                        (xq_d, xk_d, xv_d, wq_d, wk_d, wv_d,
                         bq_d, bk_d, bv_d, wo_d, bo_d, mt_d, out_d))

    nc.compile()
    return nc


def _build_body(nc, tc, bass, mybir, tile, drams):
    from contextlib import ExitStack

    FP = mybir.dt.float32
    BF = mybir.dt.bfloat16
    AL = mybir.AluOpType
    AF = mybir.ActivationFunctionType
    (xq_d, xk_d, xv_d, wq_d, wk_d, wv_d,
     bq_d, bk_d, bv_d, wo_d, bo_d, mt_d, out_d) = drams

    def apx(t, off, *dn):
        # AP over tile t: partition dim + given (stride, count) free dims
        a = t[:]
        return bass.AP(tensor=a.tensor, offset=a.offset + off,
                       ap=[list(a.ap[0])] + [[s, n] for s, n in dn])

    DMAQ = [nc.sync, nc.scalar, nc.gpsimd]

    with ExitStack() as ctx:
        persist = ctx.enter_context(tc.tile_pool(name="persist", bufs=1))
        qt = persist.tile([128, 8, C], BF)            # Q^T: [f'%128, f'//128, t]
        kt = persist.tile([128, 8, C], BF)            # K^T
        vt = persist.tile([128, 8, NH, HD + 1], BF)   # V: [t%128, t//128, h, d'] + ones
        ident = persist.tile([128, 128], FP)
        ident_b = persist.tile([128, 128], BF)
        ident_made = []

        def get_ident_b():
            # lazy: keep the identity build off the DVE queue at t=0
            if not ident_made:
                from concourse.masks import make_identity
                make_identity(nc, ident)
                nc.vector.tensor_copy(out=ident_b, in_=ident)
                ident_made.append(True)
            return ident_b

        ones_t = persist.tile([65, 256], BF)
        nc.gpsimd.memset(ones_t, 1.0)
        bias3 = persist.tile([65, C], BF)   # bias rows at partitions 0/32/64
        nc.gpsimd.dma_start(out=bias3[0:1, :], in_=bq_d[:])
        nc.gpsimd.dma_start(out=bias3[32:33, :], in_=bk_d[:])
        nc.gpsimd.dma_start(out=bias3[64:65, :], in_=bv_d[:])

        # ---------------- conv phase: Winograd F(2x2,3x3) ----------------
        # pools shared across the three convs so conv N+1's input load and
        # transforms overlap conv N's GEMM tail (xt/y share a 2-deep tag)
        cctx = ctx.enter_context(ExitStack())
        xpool = cctx.enter_context(tc.tile_pool(name="xp", bufs=2))
        vpool = cctx.enter_context(tc.tile_pool(name="vp", bufs=1))
        tpool = cctx.enter_context(tc.tile_pool(name="tp", bufs=4))
        wpool = cctx.enter_context(tc.tile_pool(name="wp", bufs=3))
        dpool = cctx.enter_context(tc.tile_pool(name="dp", bufs=5))
        ppool = cctx.enter_context(tc.tile_pool(name="pp", bufs=1))
        epool = cctx.enter_context(tc.tile_pool(name="ep", bufs=1))
        pspool = cctx.enter_context(tc.tile_pool(name="psc", bufs=3, space="PSUM"))
        psT = cctx.enter_context(tc.tile_pool(name="psT", bufs=2, space="PSUM"))

        def conv(ci, xd, wd, sink, dst):
            if True:
                xt = xpool.tile([128, 8, XL], BF, tag="xy", name="xt")
                for icc in range(8):
                    # conv 0: keep x off the Pool queue (busy with memsets at
                    # t=0); later convs: spread over all three queues since
                    # sync/scalar are deep in the previous conv's weight loads
                    DMAQ[icc % 2 if ci == 0 else icc % 3].dma_start(
                        out=xt[:, icc], in_=xd[icc * 128:(icc + 1) * 128, :])

                # stage B: column combos (eo-deinterleaved, stride-1 reads)
                # layout in xt free dim: [icc(1156)][eo(578)][y(17)][x(1)]
                def xap(eo, x0, i0, ni):
                    return apx(xt, i0 * XL + eo * 578 + x0,
                               (XL, ni), (17, 34), (1, 16))

                B_OPS = [((0, 0), (0, 1), AL.subtract),
                         ((1, 0), (0, 1), AL.add),
                         ((0, 1), (1, 0), AL.subtract),
                         ((1, 0), (1, 1), AL.subtract)]

                pa = {}  # stage D outputs P_aj: [128, 8 och, 256] bf16
                for j in range(4):
                    vj = vpool.tile([128, 8, 544], FP, tag="v", name=f"vj{j}")
                    (e0, x0), (e1, x1), bop = B_OPS[j]
                    # split by icc groups so work can start before full x lands
                    # (pairs at the very start of the program, halves after)
                    ng = 2 if (ci == 0 and j == 0) else 4
                    for ih in range(8 // ng):
                        nc.vector.tensor_tensor(
                            out=apx(vj, ih * ng * 544, (544, ng), (16, 34), (1, 16)),
                            in0=xap(e0, x0, ng * ih, ng),
                            in1=xap(e1, x1, ng * ih, ng), op=bop)

                    # stage C: row combos (stride-2 rows, packed x), fp32->bf16
                    tj = [tpool.tile([128, 8, 256], BF, tag="t", name=f"t{j}{i}")
                          for i in range(4)]

                    def vrow(r0, i0, ni):
                        return apx(vj, i0 * 544 + r0 * 16,
                                   (544, ni), (32, 16), (1, 16))

                    def tout(i, i0, ni):
                        return apx(tj[i], i0 * 256, (256, ni), (16, 16), (1, 16))

                    C_OPS = [(0, 2, AL.subtract), (1, 2, AL.add),
                             (2, 1, AL.subtract), (1, 3, AL.subtract)]
                    for i in range(4):
                        r0, r1, cop = C_OPS[i]
                        for ih in range(8 // ng):
                            nc.vector.tensor_tensor(
                                out=tout(i, ng * ih, ng),
                                in0=vrow(r0, ng * ih, ng),
                                in1=vrow(r1, ng * ih, ng), op=cop)

                    p0 = ppool.tile([128, 8, 256], BF, name=f"p0{j}")
                    p1 = ppool.tile([128, 8, 256], BF, name=f"p1{j}")
                    pa[(0, j)] = p0
                    pa[(1, j)] = p1
                    for h in range(2):
                        ps = []
                        for i in range(4):
                            ij = i * 4 + j
                            wt = wpool.tile([128, 8, 4, 128], BF, tag="w", name="wt")
                            # conv 0, first plane: the Pool queue is otherwise
                            # empty at t=0 and sync/scalar carry the x chunks
                            # the first GEMM is also waiting on
                            wq_i = 2 if (ci == 0 and ij == 0) else (2 * ij + h) % 3
                            DMAQ[wq_i].dma_start(out=wt, in_=wd[ij, h])
                            p = pspool.tile([128, 1024], FP, tag="ps", name="ph")
                            ps.append(p)
                            bias_here = (i == 1 and j == 1)
                            for oc4 in range(4):
                                off = oc4 * 256
                                for icc in range(8):
                                    nc.tensor.matmul(
                                        p[:, off:off + 256],
                                        wt[:, icc, oc4], tj[i][:, icc],
                                        start=(icc == 0),
                                        stop=(icc == 7 and not bias_here))
                                if bias_here:
                                    och = 4 * h + oc4
                                    nc.tensor.matmul(
                                        p[:, off:off + 256],
                                        bias3[32 * ci:32 * ci + 1,
                                              och * 128:(och + 1) * 128],
                                        ones_t[32 * ci:32 * ci + 1, :],
                                        start=False, stop=True)
                            # interleave stage D with the GEMMs to bound PSUM
                            # liveness.  A TensorTensor may read at most ONE
                            # operand from PSUM: drain m1/m2 to SBUF fp32 via
                            # ACT, combine m0/m3 straight out of PSUM.
                            if i == 1:
                                m1s = dpool.tile([128, 1024], FP, tag="td", name="m1s")
                                nc.scalar.copy(out=m1s, in_=ps[1])
                                t01 = dpool.tile([128, 1024], FP, tag="td", name="t01")
                                nc.vector.tensor_tensor(
                                    out=t01, in0=ps[0], in1=m1s, op=AL.add)
                            elif i == 2:
                                m2s = dpool.tile([128, 1024], FP, tag="td", name="m2s")
                                nc.scalar.copy(out=m2s, in_=ps[2])
                                nc.vector.tensor_tensor(
                                    out=p0[:, 4 * h:4 * h + 4].rearrange(
                                        "p a b -> p (a b)"),
                                    in0=t01, in1=m2s, op=AL.add)
                                t12 = dpool.tile([128, 1024], FP, tag="td", name="t12")
                                nc.vector.tensor_tensor(
                                    out=t12, in0=m1s, in1=m2s, op=AL.subtract)
                            elif i == 3:
                                nc.vector.tensor_tensor(
                                    out=p1[:, 4 * h:4 * h + 4].rearrange(
                                        "p a b -> p (a b)"),
                                    in0=t12, in1=ps[3], op=AL.subtract)



                # stage E (Pool): y(a,b): b=0: P_a0+P_a1+P_a2 ; b=1: P_a1-P_a2-P_a3
                if sink == "qk":
                    y = xpool.tile([128, 8, C], BF, tag="xy", name="yall")

                    def eout(a, b):
                        return apx(y, a * 32 + b * 16, (C, 8), (64, 16), (1, 16))
                else:
                    def eout(a, b):
                        return apx(vt, a * 32 + b * 16, (NH * 65, 8), (65, 16), (1, 16))

                for a in range(2):
                    te = epool.tile([128, 8, 256], FP, tag="te", name=f"te{a}")
                    nc.gpsimd.tensor_tensor(out=te, in0=pa[(a, 0)], in1=pa[(a, 1)], op=AL.add)
                    nc.gpsimd.tensor_tensor(out=eout(a, 0), in0=te, in1=pa[(a, 2)], op=AL.add)
                    te2 = epool.tile([128, 8, 256], FP, tag="te", name=f"te2{a}")
                    nc.gpsimd.tensor_tensor(out=te2, in0=pa[(a, 1)], in1=pa[(a, 2)], op=AL.subtract)
                    nc.gpsimd.tensor_tensor(out=eout(a, 1), in0=te2, in1=pa[(a, 3)], op=AL.subtract)

                if sink == "qk":
                    # PE transposes double as gap-fillers: at each conv
                    # boundary the PE would otherwise idle behind the DVE
                    # transform backlog (DMA-XBAR transposes measured WORSE)
                    idb = get_ident_b()
                    for och in range(8):
                        for fcc in range(8):
                            pt_ps = psT.tile([128, 128], BF, tag="t", name="ptps")
                            nc.tensor.transpose(
                                pt_ps, y[:, och, fcc * 128:(fcc + 1) * 128],
                                idb)
                            nc.scalar.copy(
                                out=dst[:, fcc, och * 128:(och + 1) * 128],
                                in_=pt_ps)

        conv(0, xq_d, wq_d, "qk", qt)
        conv(1, xk_d, wk_d, "qk", kt)
        nc.gpsimd.memset(vt, 1.0)   # ones column survives stage E writes
        conv(2, xv_d, wv_d, "v", None)
        cctx.close()

        # ---------------- attention + output linear ----------------
        with ExitStack() as actx:
            apool = actx.enter_context(tc.tile_pool(name="ap", bufs=1))
            mtb = apool.tile([128, 8, C], BF)
            for skc in range(8):
                DMAQ[skc % 3].dma_start(
                    out=mtb[:, skc], in_=mt_d[skc * 128:(skc + 1) * 128, :])
            wos = [apool.tile([128, C], BF, name=f"wos{i}") for i in range(8)]
            for fc in range(8):
                nc.sync.dma_start(out=wos[fc], in_=wo_d[fc * 128:(fc + 1) * 128, :])
            ot = apool.tile([128, 8, C], BF)   # O^T: [f'%128, f'//128, t]

            with tc.tile_pool(name="ptp", bufs=8) as ptp, \
                    tc.tile_pool(name="smallp", bufs=3) as smallp, \
                    tc.tile_pool(name="dscp", bufs=4, space="DRAM") as dscp, \
                    tc.tile_pool(name="psS", bufs=2, space="PSUM") as psS, \
                    tc.tile_pool(name="psO", bufs=1, space="PSUM") as psO:
                for fc in range(8):
                    po = {}
                    for hh in (2 * fc, 2 * fc + 1):
                        po[hh] = psO.tile([65, C], FP, tag=f"o{hh % 2}", name=f"po{hh}")
                    for tkc in range(8):
                        for hh in (2 * fc, 2 * fc + 1):
                            pb = 64 * (hh % 2)
                            s_ps = psS.tile([128, C], FP, tag="s", name="sps")
                            for qh in range(2):
                                nc.tensor.matmul(
                                    s_ps[:, qh * 512:(qh + 1) * 512],
                                    kt[pb:pb + 64, fc, tkc * 128:(tkc + 1) * 128],
                                    qt[pb:pb + 64, fc, qh * 512:(qh + 1) * 512],
                                    start=True, stop=True)
                            pte = ptp.tile([128, C], BF, tag="pe", name="pte")
                            nc.scalar.activation(
                                out=pte, in_=s_ps, func=AF.Exp, scale=0.125)
                            ptm = ptp.tile([128, C], BF, tag="pm", name="ptm")
                            nc.vector.tensor_tensor(
                                out=ptm, in0=pte, in1=mtb[:, tkc], op=AL.mult)
                            for qh in range(2):
                                nc.tensor.matmul(
                                    po[hh][:, qh * 512:(qh + 1) * 512],
                                    vt[:, tkc, hh],
                                    ptm[:, qh * 512:(qh + 1) * 512],
                                    start=(tkc == 0), stop=(tkc == 7))
                    for hh in (2 * fc, 2 * fc + 1):
                        pb = 64 * (hh % 2)
                        rden = smallp.tile([1, C], FP, tag="rd", name="rden")
                        obuf = smallp.tile([64, C], BF, tag="ob", name="obuf")
                        # release po quickly: these two gate the PSUM banks
                        # the next head pair needs
                        with tc.high_priority(offset=40):
                            nc.vector.reciprocal(out=rden, in_=po[hh][64:65, :])
                            nc.vector.tensor_copy(out=obuf, in_=po[hh][0:64, :])
                        dsc = dscp.tile([1, C], FP, tag="d", name="dsc")
                        nc.gpsimd.dma_start(out=dsc, in_=rden)
                        rbs = smallp.tile([64, C], FP, tag="rb", name="rbs")
                        dap = dsc[0:1, :]
                        nc.gpsimd.dma_start(out=rbs, in_=bass.AP(
                            tensor=dap.tensor, offset=dap.offset,
                            ap=[[0, 64]] + list(dap.ap)[1:]))
                        nc.gpsimd.tensor_tensor(
                            out=ot[pb:pb + 64, fc, :], in0=obuf, in1=rbs, op=AL.mult)

            # out^T[j, t] = Wo O^T + bo  (bias per-partition in this form)
            bob = apool.tile([128, 8], FP)
            nc.gpsimd.dma_start(out=bob, in_=bo_d[:].rearrange("(a p) -> p a", p=128))
            with tc.tile_pool(name="stg", bufs=2) as stg, \
                    tc.tile_pool(name="psL", bufs=2, space="PSUM") as psL:
                for jc in range(8):
                    pls = psL.tile([128, C], FP, tag="l", name="psl")
                    for fc in range(8):
                        for th in range(2):
                            nc.tensor.matmul(
                                pls[:, th * 512:(th + 1) * 512],
                                wos[fc][:, jc * 128:(jc + 1) * 128],
                                ot[:, fc, th * 512:(th + 1) * 512],
                                start=(fc == 0), stop=(fc == 7))
                    so = stg.tile([128, C], FP, tag="so", name="so")
                    nc.vector.tensor_scalar(
                        out=so, in0=pls, scalar1=bob[:, jc:jc + 1], scalar2=None,
                        op0=AL.add)
                    for sh in range(2):
                        DMAQ[(2 * jc + sh) % 3].dma_start(
                            out=out_d[jc * 128:(jc + 1) * 128,
                                      sh * 512:(sh + 1) * 512],
                            in_=so[:, sh * 512:(sh + 1) * 512])


_G = np.array([[1, 0, 0], [.5, .5, .5], [.5, -.5, .5], [0, 0, 1]], np.float32)


def _prep_x(x):
    # [C, 32, 32] f32 -> [C, 1156] bf16: pad to 34x34, deinterleave columns
    import ml_dtypes
    xp = np.zeros((C, 34, 34), np.float32)
    xp[:, 1:33, 1:33] = x
    arr = np.stack([xp[:, :, 0::2], xp[:, :, 1::2]], axis=1)  # [C, 2, 34, 17]
    return np.ascontiguousarray(
        arr.reshape(C, XL).astype(ml_dtypes.bfloat16))


def _prep_w(W):
    # [O, I, 3, 3] -> [16, 2, 128, 8, 4, 128] bf16: U = G W G^T per (oc, ic)
    import ml_dtypes
    U = np.einsum('iu,ocuv,jv->ijoc', _G, np.asarray(W, np.float32), _G)
    U7 = U.reshape(4, 4, 2, 4, 128, 8, 128)     # [i, j, half, oc4, oc, icc, icp]
    out = U7.transpose(0, 1, 2, 6, 5, 3, 4)     # [i, j, half, icp, icc, oc4, oc]
    return np.ascontiguousarray(
        out.reshape(16, 2, 128, 8, 4, 128).astype(ml_dtypes.bfloat16))


def get_program(reps=1):
    key = ("nc", reps)
    if key not in _CACHE:
        _CACHE[key] = _build_program(reps)
    return _CACHE[key]


def make_in_maps(q, k, v, Wq, bq, Wk, bk, Wv, bv, Wo, bo, mask):
    import ml_dtypes
    bf = ml_dtypes.bfloat16
    wq = _prep_w(np.asarray(Wq))
    wk = _prep_w(np.asarray(Wk))
    wv = _prep_w(np.asarray(Wv))
    wo = np.ascontiguousarray(np.asarray(Wo, np.float32).T[_PERM].astype(bf))
    bq_, bk_, bv_ = (np.ascontiguousarray(np.asarray(b, np.float32)
                                          .reshape(1, C).astype(bf))
                     for b in (bq, bk, bv))
    bo_ = np.ascontiguousarray(np.asarray(bo), dtype=np.float32)
    in_maps = []
    for b in range(B):
        in_maps.append({
            "xq": _prep_x(np.asarray(q[b]).reshape(C, 32, 32)),
            "xk": _prep_x(np.asarray(k[b]).reshape(C, 32, 32)),
            "xv": _prep_x(np.asarray(v[b]).reshape(C, 32, 32)),
            "wq": wq, "wk": wk, "wv": wv, "wo": wo,
            "bq": bq_, "bk": bk_, "bv": bv_, "bo": bo_,
            "mt": np.ascontiguousarray(np.asarray(mask[b]).T.astype(bf)),
        })
    return in_maps


def run(inputs, trace=False, **kw):
    from concourse.bass_utils import run_bass_kernel_spmd

    nc = get_program()
    in_maps = make_in_maps(**inputs)
    res = run_bass_kernel_spmd(nc, in_maps, list(range(B)), trace=trace, **kw)
    # device computes out^T [j, t]; transpose back on host
    out = np.stack([np.asarray(res.results[i]["out"]).T for i in range(B)], axis=0)
    return out, res


def kernel(**inputs) -> np.ndarray:
    out, _ = run(inputs, trace=False)
    return out


# revision 93
# speedup vs baseline: 1.0032x; 1.0032x over previous
"""Trainium2 Bass kernel for nn_MultiHeadAttn (conv-QKV multi-head attention).

Sharding: pure data parallelism over batch B=8 -> one batch item per NeuronCore.

Per-core pipeline (matmuls bf16; cost model charges N_out_cols x 1 cycle):
  - 3x3 SAME convs via Winograd F(2x2,3x3): 16 transformed planes, each a
    [1024ic x 1024oc] GEMM over 256 tiles -> 16*8*8 matmuls of N=256 per conv
    (2.25x fewer PE columns than direct conv).  Weights host-transformed
    (G W G^T) to bf16.
  - Precision: conv error is amplified ~4x through exp(logits/8) in the
    attention, so the transform chain keeps fp32 intermediates everywhere;
    the only roundings are x->bf16, T->bf16, U->bf16, y->bf16.  Input
    transform: stage B (columns, on host-deinterleaved even/odd planes,
    all reads stride-1) bf16->fp32, stage C (rows) fp32->bf16 on DVE.
    Output transform stage D runs on DVE straight out of PSUM with fp32
    temps into bf16 P; stage E (fp32 temps -> bf16 y) runs on Pool.
  - Conv bias folded in as a K=1 matmul into the (i=1,j=1) plane (its output
    transform coefficient is +1 for every output pixel).
  - Feature pixels are stored in a head-preserving permutation
    f' = ty*64 + a*32 + b*16 + tx (head == tile row ty), applied
    consistently to Q/K/V and Wo's input dim, so stage E writes stay packed.
  - Attention per (head, k-chunk): S^T = K Q^T (bf16), exp on ACT with
    scale=1/8 straight from PSUM to bf16, mask applied POST-exp as a bf16
    multiply on DVE (mask in {0,1}), PV matmul bf16 with a ones column
    appended to V so softmax denominators fall out of the PV matmul.
  - Normalization: fp32 reciprocal of the denominator row, DRAM-bounce
    partition-broadcast, multiply into bf16 O^T staging.
  - Output linear computed TRANSPOSED (out^T[j,t] = Wo O^T) so the bias is
    per-partition; host transposes the result back (layout only).
Host-side work is layout/cast plus weight transform (weight preprocessing).
"""

import sys

if "/opt/trn_rl_repo" not in sys.path:
    sys.path.insert(0, "/opt/trn_rl_repo")

import numpy as np

_CACHE = {}

B = 8
C = 1024          # tokens (= conv channels)
NH = 16           # heads
HD = 64           # head dim
XL = 1156         # deinterleaved padded plane: [2 eo][34 y][17 x]

# f' permutation: perm[f'] = original f, f' = ty*64 + a*32 + b*16 + tx
_PERM = np.zeros(1024, np.int64)
for _ty in range(16):
    for _a in range(2):
        for _b in range(2):
            for _tx in range(16):
                _PERM[_ty * 64 + _a * 32 + _b * 16 + _tx] = \
                    (2 * _ty + _a) * 32 + 2 * _tx + _b


def _build_program(reps=1):
    from contextlib import ExitStack

    import concourse.bass as bass
    import concourse.mybir as mybir
    import concourse.tile as tile
    from concourse import bacc

    FP = mybir.dt.float32
    BF = mybir.dt.bfloat16

    nc = bacc.Bacc(None, target_bir_lowering=False)

    xq_d = nc.dram_tensor("xq", [C, XL], BF, kind="ExternalInput")
    xk_d = nc.dram_tensor("xk", [C, XL], BF, kind="ExternalInput")
    xv_d = nc.dram_tensor("xv", [C, XL], BF, kind="ExternalInput")
    wq_d = nc.dram_tensor("wq", [16, 2, 128, 8, 4, 128], BF, kind="ExternalInput")
    wk_d = nc.dram_tensor("wk", [16, 2, 128, 8, 4, 128], BF, kind="ExternalInput")
    wv_d = nc.dram_tensor("wv", [16, 2, 128, 8, 4, 128], BF, kind="ExternalInput")
    bq_d = nc.dram_tensor("bq", [1, C], BF, kind="ExternalInput")
    bk_d = nc.dram_tensor("bk", [1, C], BF, kind="ExternalInput")
    bv_d = nc.dram_tensor("bv", [1, C], BF, kind="ExternalInput")
    wo_d = nc.dram_tensor("wo", [C, C], BF, kind="ExternalInput")   # Wo^T[f', j]
    bo_d = nc.dram_tensor("bo", [C], FP, kind="ExternalInput")
    mt_d = nc.dram_tensor("mt", [C, C], BF, kind="ExternalInput")   # mask^T [s, t]
    out_d = nc.dram_tensor("out", [C, C], FP, kind="ExternalOutput")  # out^T [j, t]

    with ExitStack() as ctx:
        tc = ctx.enter_context(tile.TileContext(nc))
        for _rep in range(reps):
            _build_body(nc, tc, bass, mybir, tile,
                        (xq_d, xk_d, xv_d, wq_d, wk_d, wv_d,
                         bq_d, bk_d, bv_d, wo_d, bo_d, mt_d, out_d))

    nc.compile()
    return nc


def _build_body(nc, tc, bass, mybir, tile, drams):
    from contextlib import ExitStack

    FP = mybir.dt.float32
    BF = mybir.dt.bfloat16
    AL = mybir.AluOpType
    AF = mybir.ActivationFunctionType
    (xq_d, xk_d, xv_d, wq_d, wk_d, wv_d,
     bq_d, bk_d, bv_d, wo_d, bo_d, mt_d, out_d) = drams

    def apx(t, off, *dn):
        # AP over tile t: partition dim + given (stride, count) free dims
        a = t[:]
        return bass.AP(tensor=a.tensor, offset=a.offset + off,
                       ap=[list(a.ap[0])] + [[s, n] for s, n in dn])

    DMAQ = [nc.sync, nc.scalar, nc.gpsimd]

    with ExitStack() as ctx:
        persist = ctx.enter_context(tc.tile_pool(name="persist", bufs=1))
        qt = persist.tile([128, 8, C], BF)            # Q^T: [f'%128, f'//128, t]
        kt = persist.tile([128, 8, C], BF)            # K^T
        vt = persist.tile([128, 8, NH, HD + 1], BF)   # V: [t%128, t//128, h, d'] + ones
        ident = persist.tile([128, 128], FP)
        ident_b = persist.tile([128, 128], BF)
        ident_made = []

        def get_ident_b():
            # lazy: keep the identity build off the DVE queue at t=0
            if not ident_made:
                from concourse.masks import make_identity
                make_identity(nc, ident)
                nc.vector.tensor_copy(out=ident_b, in_=ident)
                ident_made.append(True)
            return ident_b

        ones_t = persist.tile([65, 256], BF)
        nc.gpsimd.memset(ones_t, 1.0)
        bias3 = persist.tile([65, C], BF)   # bias rows at partitions 0/32/64

        # ---------------- conv phase: Winograd F(2x2,3x3) ----------------
        # pools shared across the three convs so conv N+1's input load and
        # transforms overlap conv N's GEMM tail (xt/y share a 2-deep tag)
        cctx = ctx.enter_context(ExitStack())
        xpool = cctx.enter_context(tc.tile_pool(name="xp", bufs=2))
        vpool = cctx.enter_context(tc.tile_pool(name="vp", bufs=1))
        tpool = cctx.enter_context(tc.tile_pool(name="tp", bufs=4))
        wpool = cctx.enter_context(tc.tile_pool(name="wp", bufs=3))
        dpool = cctx.enter_context(tc.tile_pool(name="dp", bufs=5))
        ppool = cctx.enter_context(tc.tile_pool(name="pp", bufs=1))
        epool = cctx.enter_context(tc.tile_pool(name="ep", bufs=1))
        pspool = cctx.enter_context(tc.tile_pool(name="psc", bufs=3, space="PSUM"))
        psT = cctx.enter_context(tc.tile_pool(name="psT", bufs=2, space="PSUM"))

        def conv(ci, xd, wd, sink, dst):
            if True:
                xt = xpool.tile([128, 8, XL], BF, tag="xy", name="xt")
                for icc in range(8):
                    # conv 0: keep x off the Pool queue (busy with memsets at
                    # t=0); later convs: spread over all three queues since
                    # sync/scalar are deep in the previous conv's weight loads
                    DMAQ[icc % 2 if ci == 0 else icc % 3].dma_start(
                        out=xt[:, icc], in_=xd[icc * 128:(icc + 1) * 128, :])

                # stage B: column combos (eo-deinterleaved, stride-1 reads)
                # layout in xt free dim: [icc(1156)][eo(578)][y(17)][x(1)]
                def xap(eo, x0, i0, ni):
                    return apx(xt, i0 * XL + eo * 578 + x0,
                               (XL, ni), (17, 34), (1, 16))

                B_OPS = [((0, 0), (0, 1), AL.subtract),
                         ((1, 0), (0, 1), AL.add),
                         ((0, 1), (1, 0), AL.subtract),
                         ((1, 0), (1, 1), AL.subtract)]

                pa = {}  # stage D outputs P_aj: [128, 8 och, 256] bf16
                for j in range(4):
                    vj = vpool.tile([128, 8, 544], FP, tag="v", name=f"vj{j}")
                    (e0, x0), (e1, x1), bop = B_OPS[j]
                    # split by icc groups so work can start before full x lands
                    # (pairs at the very start of the program, halves after)
                    ng = 2 if (ci == 0 and j == 0) else 4
                    for ih in range(8 // ng):
                        nc.vector.tensor_tensor(
                            out=apx(vj, ih * ng * 544, (544, ng), (16, 34), (1, 16)),
                            in0=xap(e0, x0, ng * ih, ng),
                            in1=xap(e1, x1, ng * ih, ng), op=bop)

                    # stage C: row combos (stride-2 rows, packed x), fp32->bf16
                    tj = [tpool.tile([128, 8, 256], BF, tag="t", name=f"t{j}{i}")
                          for i in range(4)]

                    def vrow(r0, i0, ni):
                        return apx(vj, i0 * 544 + r0 * 16,
                                   (544, ni), (32, 16), (1, 16))

                    def tout(i, i0, ni):
                        return apx(tj[i], i0 * 256, (256, ni), (16, 16), (1, 16))

                    C_OPS = [(0, 2, AL.subtract), (1, 2, AL.add),
                             (2, 1, AL.subtract), (1, 3, AL.subtract)]
                    for i in range(4):
                        r0, r1, cop = C_OPS[i]
                        for ih in range(8 // ng):
                            nc.vector.tensor_tensor(
                                out=tout(i, ng * ih, ng),
                                in0=vrow(r0, ng * ih, ng),
                                in1=vrow(r1, ng * ih, ng), op=cop)

                    p0 = ppool.tile([128, 8, 256], BF, name=f"p0{j}")
                    p1 = ppool.tile([128, 8, 256], BF, name=f"p1{j}")
                    pa[(0, j)] = p0
                    pa[(1, j)] = p1
                    for h in range(2):
                        ps = []
                        for i in range(4):
                            ij = i * 4 + j
                            wt = wpool.tile([128, 8, 4, 128], BF, tag="w", name="wt")
                            # conv 0, first plane: the Pool queue is otherwise
                            # empty at t=0 and sync/scalar carry the x chunks
                            # the first GEMM is also waiting on
                            wq_i = 2 if (ci == 0 and ij == 0) else (2 * ij + h) % 3
                            DMAQ[wq_i].dma_start(out=wt, in_=wd[ij, h])
                            p = pspool.tile([128, 1024], FP, tag="ps", name="ph")
                            ps.append(p)
                            bias_here = (i == 1 and j == 1)
                            for oc4 in range(4):
                                off = oc4 * 256
                                for icc in range(8):
                                    nc.tensor.matmul(
                                        p[:, off:off + 256],
                                        wt[:, icc, oc4], tj[i][:, icc],
                                        start=(icc == 0),
                                        stop=(icc == 7 and not bias_here))
                                if bias_here:
                                    och = 4 * h + oc4
                                    nc.tensor.matmul(
                                        p[:, off:off + 256],
                                        bias3[32 * ci:32 * ci + 1,
                                              och * 128:(och + 1) * 128],
                                        ones_t[32 * ci:32 * ci + 1, :],
                                        start=False, stop=True)
                            # interleave stage D with the GEMMs to bound PSUM
                            # liveness.  A TensorTensor may read at most ONE
                            # operand from PSUM: drain m1/m2 to SBUF fp32 via
                            # ACT, combine m0/m3 straight out of PSUM.
                            if i == 1:
                                m1s = dpool.tile([128, 1024], FP, tag="td", name="m1s")
                                nc.scalar.copy(out=m1s, in_=ps[1])
                                t01 = dpool.tile([128, 1024], FP, tag="td", name="t01")
                                nc.vector.tensor_tensor(
                                    out=t01, in0=ps[0], in1=m1s, op=AL.add)
                            elif i == 2:
                                m2s = dpool.tile([128, 1024], FP, tag="td", name="m2s")
                                nc.scalar.copy(out=m2s, in_=ps[2])
                                nc.vector.tensor_tensor(
                                    out=p0[:, 4 * h:4 * h + 4].rearrange(
                                        "p a b -> p (a b)"),
                                    in0=t01, in1=m2s, op=AL.add)
                                t12 = dpool.tile([128, 1024], FP, tag="td", name="t12")
                                nc.vector.tensor_tensor(
                                    out=t12, in0=m1s, in1=m2s, op=AL.subtract)
                            elif i == 3:
                                nc.vector.tensor_tensor(
                                    out=p1[:, 4 * h:4 * h + 4].rearrange(
                                        "p a b -> p (a b)"),
                                    in0=t12, in1=ps[3], op=AL.subtract)



                # stage E (Pool): y(a,b): b=0: P_a0+P_a1+P_a2 ; b=1: P_a1-P_a2-P_a3
                if sink == "qk":
                    y = xpool.tile([128, 8, C], BF, tag="xy", name="yall")

                    def eout(a, b):
                        return apx(y, a * 32 + b * 16, (C, 8), (64, 16), (1, 16))
                else:
                    def eout(a, b):
                        return apx(vt, a * 32 + b * 16, (NH * 65, 8), (65, 16), (1, 16))

                for a in range(2):
                    te = epool.tile([128, 8, 256], FP, tag="te", name=f"te{a}")
                    nc.gpsimd.tensor_tensor(out=te, in0=pa[(a, 0)], in1=pa[(a, 1)], op=AL.add)
                    nc.gpsimd.tensor_tensor(out=eout(a, 0), in0=te, in1=pa[(a, 2)], op=AL.add)
                    te2 = epool.tile([128, 8, 256], FP, tag="te", name=f"te2{a}")
                    nc.gpsimd.tensor_tensor(out=te2, in0=pa[(a, 1)], in1=pa[(a, 2)], op=AL.subtract)
                    nc.gpsimd.tensor_tensor(out=eout(a, 1), in0=te2, in1=pa[(a, 3)], op=AL.subtract)

                if sink == "qk":
                    # PE transposes double as gap-fillers: at each conv
                    # boundary the PE would otherwise idle behind the DVE
                    # transform backlog (DMA-XBAR transposes measured WORSE)
                    idb = get_ident_b()
                    for och in range(8):
                        for fcc in range(8):
                            pt_ps = psT.tile([128, 128], BF, tag="t", name="ptps")
                            nc.tensor.transpose(
                                pt_ps, y[:, och, fcc * 128:(fcc + 1) * 128],
                                idb)
                            nc.scalar.copy(
                                out=dst[:, fcc, och * 128:(och + 1) * 128],
                                in_=pt_ps)

        conv(0, xq_d, wq_d, "qk", qt)
        # bias loads issued AFTER conv 0's emission: they'd otherwise sit
        # ahead of the first weight tile on the gpsimd SWDGE queue (~1us
        # generation each) and delay the first GEMM; first read is ~40us in
        nc.gpsimd.dma_start(out=bias3[0:1, :], in_=bq_d[:])
        nc.gpsimd.dma_start(out=bias3[32:33, :], in_=bk_d[:])
        nc.gpsimd.dma_start(out=bias3[64:65, :], in_=bv_d[:])
        conv(1, xk_d, wk_d, "qk", kt)
        nc.gpsimd.memset(vt, 1.0)   # ones column survives stage E writes
        conv(2, xv_d, wv_d, "v", None)
        cctx.close()

        # ---------------- attention + output linear ----------------
        with ExitStack() as actx:
            apool = actx.enter_context(tc.tile_pool(name="ap", bufs=1))
            mtb = apool.tile([128, 8, C], BF)
            for skc in range(8):
                DMAQ[skc % 3].dma_start(
                    out=mtb[:, skc], in_=mt_d[skc * 128:(skc + 1) * 128, :])
            wos = [apool.tile([128, C], BF, name=f"wos{i}") for i in range(8)]
            for fc in range(8):
                nc.sync.dma_start(out=wos[fc], in_=wo_d[fc * 128:(fc + 1) * 128, :])
            ot = apool.tile([128, 8, C], BF)   # O^T: [f'%128, f'//128, t]

            with tc.tile_pool(name="ptp", bufs=8) as ptp, \
                    tc.tile_pool(name="smallp", bufs=3) as smallp, \
                    tc.tile_pool(name="dscp", bufs=4, space="DRAM") as dscp, \
                    tc.tile_pool(name="psS", bufs=2, space="PSUM") as psS, \
                    tc.tile_pool(name="psO", bufs=1, space="PSUM") as psO:
                for fc in range(8):
                    po = {}
                    for hh in (2 * fc, 2 * fc + 1):
                        po[hh] = psO.tile([65, C], FP, tag=f"o{hh % 2}", name=f"po{hh}")
                    for tkc in range(8):
                        for hh in (2 * fc, 2 * fc + 1):
                            pb = 64 * (hh % 2)
                            s_ps = psS.tile([128, C], FP, tag="s", name="sps")
                            for qh in range(2):
                                nc.tensor.matmul(
                                    s_ps[:, qh * 512:(qh + 1) * 512],
                                    kt[pb:pb + 64, fc, tkc * 128:(tkc + 1) * 128],
                                    qt[pb:pb + 64, fc, qh * 512:(qh + 1) * 512],
                                    start=True, stop=True)
                            pte = ptp.tile([128, C], BF, tag="pe", name="pte")
                            nc.scalar.activation(
                                out=pte, in_=s_ps, func=AF.Exp, scale=0.125)
                            ptm = ptp.tile([128, C], BF, tag="pm", name="ptm")
                            nc.vector.tensor_tensor(
                                out=ptm, in0=pte, in1=mtb[:, tkc], op=AL.mult)
                            for qh in range(2):
                                nc.tensor.matmul(
                                    po[hh][:, qh * 512:(qh + 1) * 512],
                                    vt[:, tkc, hh],
                                    ptm[:, qh * 512:(qh + 1) * 512],
                                    start=(tkc == 0), stop=(tkc == 7))
                    for hh in (2 * fc, 2 * fc + 1):
                        pb = 64 * (hh % 2)
                        rden = smallp.tile([1, C], FP, tag="rd", name="rden")
                        obuf = smallp.tile([64, C], BF, tag="ob", name="obuf")
                        # release po quickly: these two gate the PSUM banks
                        # the next head pair needs
                        with tc.high_priority(offset=40):
                            nc.vector.reciprocal(out=rden, in_=po[hh][64:65, :])
                            nc.vector.tensor_copy(out=obuf, in_=po[hh][0:64, :])
                        dsc = dscp.tile([1, C], FP, tag="d", name="dsc")
                        nc.gpsimd.dma_start(out=dsc, in_=rden)
                        rbs = smallp.tile([64, C], FP, tag="rb", name="rbs")
                        dap = dsc[0:1, :]
                        nc.gpsimd.dma_start(out=rbs, in_=bass.AP(
                            tensor=dap.tensor, offset=dap.offset,
                            ap=[[0, 64]] + list(dap.ap)[1:]))
                        nc.gpsimd.tensor_tensor(
                            out=ot[pb:pb + 64, fc, :], in0=obuf, in1=rbs, op=AL.mult)

            # out^T[j, t] = Wo O^T + bo  (bias per-partition in this form)
            bob = apool.tile([128, 8], FP)
            nc.gpsimd.dma_start(out=bob, in_=bo_d[:].rearrange("(a p) -> p a", p=128))
            with tc.tile_pool(name="stg", bufs=2) as stg, \
                    tc.tile_pool(name="psL", bufs=2, space="PSUM") as psL:
                for jc in range(8):
                    pls = psL.tile([128, C], FP, tag="l", name="psl")
                    for fc in range(8):
                        for th in range(2):
                            nc.tensor.matmul(
                                pls[:, th * 512:(th + 1) * 512],
                                wos[fc][:, jc * 128:(jc + 1) * 128],
                                ot[:, fc, th * 512:(th + 1) * 512],
                                start=(fc == 0), stop=(fc == 7))
                    so = stg.tile([128, C], FP, tag="so", name="so")
                    nc.vector.tensor_scalar(
                        out=so, in0=pls, scalar1=bob[:, jc:jc + 1], scalar2=None,
                        op0=AL.add)
                    for sh in range(2):
                        DMAQ[(2 * jc + sh) % 3].dma_start(
                            out=out_d[jc * 128:(jc + 1) * 128,
                                      sh * 512:(sh + 1) * 512],
                            in_=so[:, sh * 512:(sh + 1) * 512])


_G = np.array([[1, 0, 0], [.5, .5, .5], [.5, -.5, .5], [0, 0, 1]], np.float32)


def _prep_x(x):
    # [C, 32, 32] f32 -> [C, 1156] bf16: pad to 34x34, deinterleave columns
    import ml_dtypes
    xp = np.zeros((C, 34, 34), np.float32)
    xp[:, 1:33, 1:33] = x
    arr = np.stack([xp[:, :, 0::2], xp[:, :, 1::2]], axis=1)  # [C, 2, 34, 17]
    return np.ascontiguousarray(
        arr.reshape(C, XL).astype(ml_dtypes.bfloat16))


def _prep_w(W):
    # [O, I, 3, 3] -> [16, 2, 128, 8, 4, 128] bf16: U = G W G^T per (oc, ic)
    import ml_dtypes
    U = np.einsum('iu,ocuv,jv->ijoc', _G, np.asarray(W, np.float32), _G)
    U7 = U.reshape(4, 4, 2, 4, 128, 8, 128)     # [i, j, half, oc4, oc, icc, icp]
    out = U7.transpose(0, 1, 2, 6, 5, 3, 4)     # [i, j, half, icp, icc, oc4, oc]
    return np.ascontiguousarray(
        out.reshape(16, 2, 128, 8, 4, 128).astype(ml_dtypes.bfloat16))


def get_program(reps=1):
    key = ("nc", reps)
    if key not in _CACHE:
        _CACHE[key] = _build_program(reps)
    return _CACHE[key]


def make_in_maps(q, k, v, Wq, bq, Wk, bk, Wv, bv, Wo, bo, mask):
    import ml_dtypes
    bf = ml_dtypes.bfloat16
    wq = _prep_w(np.asarray(Wq))
    wk = _prep_w(np.asarray(Wk))
    wv = _prep_w(np.asarray(Wv))
    wo = np.ascontiguousarray(np.asarray(Wo, np.float32).T[_PERM].astype(bf))
    bq_, bk_, bv_ = (np.ascontiguousarray(np.asarray(b, np.float32)
                                          .reshape(1, C).astype(bf))
                     for b in (bq, bk, bv))
    bo_ = np.ascontiguousarray(np.asarray(bo), dtype=np.float32)
    in_maps = []
    for b in range(B):
        in_maps.append({
            "xq": _prep_x(np.asarray(q[b]).reshape(C, 32, 32)),
            "xk": _prep_x(np.asarray(k[b]).reshape(C, 32, 32)),
            "xv": _prep_x(np.asarray(v[b]).reshape(C, 32, 32)),
            "wq": wq, "wk": wk, "wv": wv, "wo": wo,
            "bq": bq_, "bk": bk_, "bv": bv_, "bo": bo_,
            "mt": np.ascontiguousarray(np.asarray(mask[b]).T.astype(bf)),
        })
    return in_maps


def run(inputs, trace=False, **kw):
    from concourse.bass_utils import run_bass_kernel_spmd

    nc = get_program()
    in_maps = make_in_maps(**inputs)
    res = run_bass_kernel_spmd(nc, in_maps, list(range(B)), trace=trace, **kw)
    # device computes out^T [j, t]; transpose back on host
    out = np.stack([np.asarray(res.results[i]["out"]).T for i in range(B)], axis=0)
    return out, res


def kernel(**inputs) -> np.ndarray:
    out, _ = run(inputs, trace=False)
    return out
